# revision 68
# baseline (speedup 1.0000x reference)
"""Trainium2 Bass kernel for a custom transformer block.

Sharding: 8 cores = 4 batches x 2 interleaved query-chunk sets. Core (b, h)
owns query chunks {2s+h : s in 0..8} (128 tokens each) of batch b; the KV
window (last 1024 tokens) is recomputed on both cores of a batch pair. The
stride-2 interleave balances the causal-triangular attention work across the
pair and lets the score matmuls skip fully-masked key/query blocks.

v2: the attention / LN2 / MLP phases are fused into one pipeline over the
two 512-query halves. The PE de-ramps to 1.2 GHz whenever it idles >some
window (p-state), so the old phase-major schedule (exp-bound attention
phase with a sparse PE, then a dense MLP phase) ran the attention matmuls
at half clock. Here attention half 0 runs against the tail of the Q
projection, then the MLP for half 0 interleaves with attention half 1,
keeping the PE dense end-to-end and hiding the scalar-engine exp cost
under MLP matmuls. The MLP accumulates h2 directly in PSUM over all 32
ff-chunks (no SBUF accumulator) and adds bias+residual in the single
evacuation; weight blocks stream per half in snake order so the last
block of half 0 is reused by half 1.

Matmul operands are bf16 (fp32 PSUM accumulation) except half of the
MLP contractions, which run in fp8-e4m3 DoubleRow (double-pumped) mode:
w1 contraction dims 0:512 and w2 ff rows 0:1024, weights pre-scaled
(32x / 64x) into e4m3's sweet spot and rescaled at the silu / output
evacuation. That split spends the 2e-2 error budget deliberately:
measured 1.71e-2 vs 3.0e-3 all-bf16. LN stats, softmax denominators and
the residual stay fp32. LN gains/biases are folded into the weights
host-side; b2 is folded into the transposed residual input.

Key padding is free: padded keys' V rows AND their ones-column entries
are zeroed (keep = 1-pad), so both the AV numerator and the row-sum
denominator exclude them; the V bias (softmax-invariant) is added back
after normalization. The causal diagonal uses a 2D min-mask on the bf16
probabilities applied per kc-pair in one op. Scores are computed
transposed ([key, query]) with each key chunk in its own PSUM bank
(start=True zeroes whole 2KB banks); row sums ride an extra column on V
and the normalization is a per-partition multiply after the PE transpose
back to token-major.
"""
import sys
import os

if "/opt/trn_rl_repo" not in sys.path:
    sys.path.insert(0, "/opt/trn_rl_repo")

import numpy as np
import ml_dtypes

B, S, D = 4, 2048, 1024
N_HEAD = 16
D_HEAD = 64
WINDOW = 1024
D_FF = 4096
EPS = 1e-5
ISD = float(1.0 / np.sqrt(D))  # 1/32
MASKVAL = -80.0
EXPMASK = float(np.exp(-80.0))  # 1.8e-35: effectively zero, bf16-normal
KEEPVAL = 3e38
P = 128

# first live query slot for key chunk kc (strip start = 128*S_MIN[kc]);
# slot s holds query chunk 2s+h, live when kc <= 2s+h -> s >= ceil((kc-1)/2)
S_MIN = [kc // 2 for kc in range(8)]  # == ceil((kc-1)/2): [0,0,1,1,2,2,3,3]

_CACHE = {}


def _build_program():
    import concourse.bacc as bacc
    import concourse.mybir as mybir
    from concourse.tile import TileContext
    from concourse.masks import make_identity

    F32 = mybir.dt.float32
    BF16 = mybir.dt.bfloat16
    F8 = mybir.dt.float8e4
    DR = mybir.MatmulPerfMode.DoubleRow
    AF = mybir.ActivationFunctionType
    ALU = mybir.AluOpType
    AX = mybir.AxisListType

    nc = bacc.Bacc("TRN2", target_bir_lowering=False, debug=False,
                   num_devices=8)

    xin_d = nc.dram_tensor("xin", [2 * WINDOW, D], BF16,
                           kind="ExternalInput")
    wq_d = nc.dram_tensor("wq", [D, D], BF16, kind="ExternalInput")
    wkv_d = nc.dram_tensor("wkv", [D, 2 * D], BF16, kind="ExternalInput")
    # w1 pre-scaled by 32 host-side; contraction rows 0:512 as fp8 e4m3
    # (DoubleRow double-pumped), rows 512:1024 as bf16; silu rescales 1/32.
    # w2 pre-scaled by 64; ff rows 0:1024 fp8, rest bf16; evac rescales 1/64.
    w18_d = nc.dram_tensor("w18", [D // 2, D_FF], F8, kind="ExternalInput")
    w1_d = nc.dram_tensor("w1", [D // 2, D_FF], BF16, kind="ExternalInput")
    w28_d = nc.dram_tensor("w28", [D_FF // 4, D], F8, kind="ExternalInput")
    w2_d = nc.dram_tensor("w2", [3 * D_FF // 4, D], BF16,
                          kind="ExternalInput")
    # all small per-partition constants ride in one DMA:
    # [bqs 0:8 | bkvk 8:16 | b1s 16:48 | b2s 48:56 | padb 56:64]
    consts_d = nc.dram_tensor("consts", [P, 64], F32, kind="ExternalInput")
    bkvvb_d = nc.dram_tensor("bkvvb", [P, D], F32, kind="ExternalInput")
    masks_d = nc.dram_tensor("masks", [P, 2 * P], BF16, kind="ExternalInput")
    xinT_d = nc.dram_tensor("xinT", [D, WINDOW], F32, kind="ExternalInput")
    y_d = nc.dram_tensor("y", [D, WINDOW], F32, kind="ExternalOutput")

    with TileContext(nc) as tc:
        cpool = tc.alloc_tile_pool(name="const", bufs=1, side="left")
        identB = cpool.tile([P, P], BF16)
        make_identity(nc, identB[:])
        masks = cpool.tile([P, 2 * P], BF16)
        smallc = cpool.tile([P, 80], F32)
        bqs = smallc[:, 0:8]
        bkvk = smallc[:, 8:16]
        b1s = smallc[:, 16:48]
        b2s = smallc[:, 48:56]
        keepc = smallc[:, 56:64]          # 1 - key_pad_mask, dim-chunked
        onesc = smallc[:, 64:80]
        nc.vector.memset(onesc, 1.0)

        def load_consts():  # deferred so the x DMAs win the queue
            nc.sync.dma_start(smallc[:, 0:64], consts_d[:])
            nc.sync.dma_start(masks[:], masks_d[:])

        # persistent attention-phase state, allocated first so the
        # early-dying LN/projection pools sit on top of the left stack
        attnp = tc.alloc_tile_pool(name="attn", bufs=1, side="left")
        ptsA = attnp.tile([P, 2, 8, 512], BF16)
        ptsB = attnp.tile([P, 2, 8, 512], BF16)
        rinva = attnp.tile([P, 4, N_HEAD], F32)
        # one query-half of LN2'd tokens, dim-major; reused by half 1 once
        # the half-0 h1 pass has consumed it (subtile deps serialize)
        z2T = attnp.tile([P, 8, 512], BF16)
        z2T8 = attnp.tile([P, 4, 512], F8)   # fp8 copy of chunks 0-3
        ahp = tc.alloc_tile_pool(name="ahp", bufs=1, side="left")
        sbD = tc.alloc_tile_pool(name="sbD", bufs=5, side="left")
        wst = tc.alloc_tile_pool(name="wst", bufs=1, side="left")

        # ---------------- Phase B/C: LN1 + QKV projections ------------------
        # z = (x - mu) * rstd token-major (LN gain/bias folded into weights);
        # 8 PE transposes batch into one PSUM bank, one ACT copy evacuates.
        # Window tiles (8-15) first so the V/K projections overlap the LN of
        # the query half, keeping the PE dense from the start.
        zTp = tc.alloc_tile_pool(name="zT", bufs=1, side="left")
        zqT = zTp.tile([P, 8, WINDOW], BF16)
        zwT = zTp.tile([P, 8, WINDOW], BF16)
        xz = tc.alloc_tile_pool(name="xz", bufs=4, side="left")
        psB = tc.alloc_tile_pool(name="psB", bufs=3, space="PSUM")

        def ln_stats(pool, xt, tag):
            """both sums on the scalar engine (Identity / Square with
            accumulator; the DVE TENSOR_REDUCE costs 1.2us and would pace
            the vector queue); var = E[x^2] - mu^2."""
            st = pool.tile([P, 8], F32, tag="stats" + tag, name="st")
            junk = pool.tile([P, D], BF16, tag="junk" + tag, name="junk")
            musum, mu, sq = st[:, 0:1], st[:, 1:2], st[:, 2:3]
            mu2, veps, sdv, rstd = (st[:, 3:4], st[:, 4:5], st[:, 5:6],
                                    st[:, 6:7])
            nc.vector.reduce_sum(musum, xt, axis=AX.X)
            nc.scalar.activation(junk[:], xt, AF.Square, accum_out=sq)
            nc.vector.tensor_scalar_mul(mu, musum, 1.0 / D)
            nc.vector.tensor_tensor(mu2, mu, mu, op=ALU.mult)
            nc.vector.tensor_scalar(veps, sq, 1.0 / D, EPS,
                                    op0=ALU.mult, op1=ALU.add)
            nc.vector.tensor_tensor(veps, veps, mu2, op=ALU.subtract)
            nc.scalar.sqrt(sdv, veps)
            nc.vector.reciprocal(rstd, sdv)
            return mu, rstd

        def ln1_tile(t):
            xt = xz.tile([P, D], BF16, tag="x")
            nc.sync.dma_start(xt[:], xin_d[t * P:(t + 1) * P, :])
            mu, rstd = ln_stats(xz, xt[:], "1")
            z = xz.tile([P, D], BF16, tag="z")
            nc.vector.tensor_scalar(z[:], xt[:], mu, rstd,
                                    op0=ALU.subtract, op1=ALU.mult)
            batch = psB.tile([P, D], BF16, tag="tpB")
            for c in range(8):
                nc.tensor.transpose(batch[:, c * P:(c + 1) * P],
                                    z[:, c * P:(c + 1) * P], identB[:])
            dst = zqT if t < 8 else zwT
            col = (t % 8) * P
            # window tiles evacuate on scalar (vector runs the V-bias
            # evacs there); query tiles on vector (scalar runs QK evacs)
            src = batch[:].rearrange("p (c n) -> p c n", n=P)
            if t < 8:
                nc.vector.tensor_copy(dst[:, :, col:col + P], src)
            else:
                nc.scalar.copy(dst[:, :, col:col + P], src)

        qkvp = tc.alloc_tile_pool(name="qkv", bufs=1, side="right")
        qT = qkvp.tile([P, 8, WINDOW], BF16)      # q/sqrt(D), dim-major
        kT = qkvp.tile([P, 8, WINDOW], BF16)      # k, dim-major
        V = qkvp.tile([P, 8, N_HEAD * 65], BF16)  # token-major + ones col

        psC = tc.alloc_tile_pool(name="psC", bufs=2, space="PSUM")

        # broadcast V-bias rows; lives until the last trans_head, so cpool
        bkvvb = cpool.tile([P, D], F32, tag="bkvvb")

        def wload(w_d, c0):  # [D, 512] weight block, dim-chunked, one DMA
            wr = wst.tile([P, 8, 512], BF16, tag="wkres", bufs=4, name="wr")
            nc.sync.dma_start(
                wr[:], w_d.rearrange("(c p) n -> p c n", p=P)[:, :, c0:c0 + 512])
            return wr

        def v_chain(tt, vh, wvr):
            """V rows of padded keys are zeroed (keepc) so both the AV
            numerator and the ones-column row sums exclude them; the V bias
            is softmax-invariant and is added back after normalization."""
            pp = psC.tile([P, 512], F32, tag="proj")
            for kc in range(8):
                nc.tensor.matmul(
                    pp[:], zwT[:, kc, tt * P:(tt + 1) * P],
                    wvr[:, kc, :],
                    start=(kc == 0), stop=(kc == 7))
            vdst = V[:, tt, :].rearrange("p (h n) -> p h n", n=65)[
                :, vh * 8:(vh + 1) * 8, 0:64]
            nc.vector.tensor_scalar_mul(
                vdst, pp[:].rearrange("p (h n) -> p h n", n=64),
                keepc[:, tt:tt + 1])

        def kq_co(wkr, co, qh, dst, src, bias, scale, pool=None, tag="proj"):
            """one 128-dim output chunk of the K or Q projection; evac on
            DVE (the scalar engine carries the exp/silu load)"""
            pool = pool if pool is not None else psC
            pp = pool.tile([P, 512], F32, tag=tag, name="pp")
            for kc in range(8):
                nc.tensor.matmul(
                    pp[:], wkr[:, kc, (co % 4) * P:(co % 4 + 1) * P],
                    src[:, kc, qh * 512:(qh + 1) * 512],
                    start=(kc == 0), stop=(kc == 7))
            nc.vector.tensor_scalar(
                dst[:, co, qh * 512:(qh + 1) * 512], pp[:],
                scale, bias[:, co:co + 1], op0=ALU.mult, op1=ALU.add)

        # window x tiles first (they gate V/K), the window LN pipeline
        # interleaved with the V token-chunk chains, then K, then Q half 0.
        ln1_tile(8)
        ln1_tile(9)
        load_consts()
        nc.sync.dma_start(bkvvb[:], bkvvb_d[:])
        wvr0 = wload(wkv_d, D)
        for tt in range(8):
            if tt + 10 < 16:
                ln1_tile(tt + 10)
            v_chain(tt, 0, wvr0)
            if tt == 1:
                wvr1 = wload(wkv_d, D + 512)
        for tt in range(8):
            v_chain(tt, 1, wvr1)
            # ones column scaled by keep: padded keys drop out of the
            # row-sum denominator as well as the AV numerator
            nc.vector.tensor_scalar_mul(
                V[:, tt, :].rearrange("p (h n) -> p h n", n=65)[:, :, 64:65],
                onesc.rearrange("p (h n) -> p h n", n=1),
                keepc[:, tt:tt + 1])
        wkr0 = wload(wkv_d, 0)
        ln1_tile(0)
        for co in range(4):
            kq_co(wkr0, co, 0, kT, zwT, bkvk, 1.0)
            kq_co(wkr0, co, 1, kT, zwT, bkvk, 1.0)
        wkr1 = wload(wkv_d, 512)
        ln1_tile(1)
        ln1_tile(2)
        for co in range(4, 8):
            kq_co(wkr1, co, 0, kT, zwT, bkvk, 1.0)
            kq_co(wkr1, co, 1, kT, zwT, bkvk, 1.0)
        wqr0 = wload(wq_d, 0)
        ln1_tile(3)
        ln1_tile(4)
        wqr1 = wload(wq_d, 512)
        ln1_tile(5)
        ln1_tile(6)
        ln1_tile(7)

        # ---------------- fused attention + LN2 + MLP pipeline ---------------
        # half-0 dead regions: strip for kc starts at query col S_MIN[kc]*128
        for pts in (ptsA, ptsB):
            for hs in range(2):
                for kc in range(2, 8):
                    z0 = S_MIN[kc] * P
                    nc.vector.memset(pts[:, hs, kc, 0:z0], 0.0)

        # Q half 0 feeds the half-0 score strips; release psB once LN1 done
        for co in range(4):
            kq_co(wqr0, co, 0, qT, zqT, bqs, ISD)
        for co in range(4, 8):
            kq_co(wqr1, co, 0, qT, zqT, bqs, ISD)
        xz.release()
        psC.release()
        psB.release()

        scorep = tc.alloc_tile_pool(name="scorep", bufs=2, space="PSUM")
        avtp = tc.alloc_tile_pool(name="avtp", bufs=2, space="PSUM")

        def scores_exp(pts, p, half):
            """score strips for head pair (2p, 2p+1) over one query half,
            packed per kc-pair into a [P, 1024] PSUM tile, exp'd in one ACT
            op per (head, kc-pair), then the causal/diagonal 2D min-mask."""
            q0 = half * 512
            for c in range(4):
                z0 = c * P if half == 0 else 0   # S_MIN[2c] == S_MIN[2c+1]
                w = 512 - z0
                for hs in range(2):
                    h = 2 * p + hs
                    po, ch = hs * 64, p
                    # each kc gets its own 512-col (= PSUM bank) region:
                    # start=True zeroes the whole 2KB bank, so two strips
                    # must never share one
                    sp = scorep.tile([P, 2 * 512], F32, tag="s", name="sp")
                    for j in range(2):
                        kc = 2 * c + j
                        nc.tensor.matmul(
                            sp[:, j * 512:j * 512 + w],
                            kT[po:po + 64, ch, kc * P:(kc + 1) * P],
                            qT[po:po + 64, ch, q0 + z0:q0 + 512],
                            start=True, stop=True)
                    dst = pts[:, hs, 2 * c:2 * c + 2, z0:512]
                    src = sp[:].rearrange("p (two n) -> p two n",
                                          n=512)[:, :, 0:w]
                    nc.scalar.activation(dst, src, AF.Exp)
                    if half == 0:
                        # diagonal block of both kc in the pair, one op
                        mdst = pts[:, hs, 2 * c:2 * c + 2, z0:z0 + P]
                        nc.vector.tensor_tensor(
                            mdst, mdst,
                            masks[:].rearrange("p (two n) -> p two n", n=P),
                            op=ALU.min)

        oa_tiles = {}

        def av_head(pts, h, half, drain=False):
            oa_tiles[h] = sbD.tile([65, 512], BF16, tag="oa", name="oa")
            avp = avtp.tile([65, 512], F32, tag="avtp", name="avp")
            for kc in range(8):
                nc.tensor.matmul(
                    avp[:], V[:, kc, h * 65:(h + 1) * 65],
                    pts[:, h % 2, kc, 0:512],
                    start=(kc == 0), stop=(kc == 7))
            if drain:
                nc.scalar.copy(oa_tiles[h][:], avp[:])
            else:
                nc.vector.tensor_copy(oa_tiles[h][:], avp[:])

        def trans_head(h, attn_h, drain=False):
            # normalize by the keep-weighted row sum and add back the V bias
            # (softmax weights sum to 1, so the bias is additive post-AV)
            oa = oa_tiles[h]
            bv = bkvvb[:, h * 64:(h + 1) * 64]
            tpb = avtp.tile([P, 4, 66], BF16, tag="avtp", name="tpb")
            for t in range(4):
                nc.tensor.transpose(tpb[:, t, 0:65],
                                    oa[:, t * P:(t + 1) * P],
                                    identB[0:65, 0:65])
            for t in range(4):
                rinv = rinva[:, t, h:h + 1]
                nc.vector.reciprocal(rinv, tpb[:, t, 64:65])
                nc.vector.scalar_tensor_tensor(
                    attn_h[:, t, h * 64:(h + 1) * 64], tpb[:, t, 0:64],
                    rinv, bv, op0=ALU.mult, op1=ALU.add)

        def ln2_tile(attn_h, t, half):
            at = attn_h[:, t, :]
            mu, rstd = ln_stats(xz2, at, "2")
            z = xz2.tile([P, D], BF16, tag="zE")
            nc.vector.tensor_scalar(z[:], at, mu, rstd,
                                    op0=ALU.subtract, op1=ALU.mult)
            batch = avtp.tile([P, D], BF16, tag="avtp", name="psE")
            for c in range(8):
                nc.tensor.transpose(batch[:, c * P:(c + 1) * P],
                                    z[:, c * P:(c + 1) * P], identB[:])
            nc.vector.tensor_copy(z2T[:, :, t * P:(t + 1) * P],
                                  batch[:].rearrange("p (c n) -> p c n", n=P))
            nc.vector.tensor_copy(
                z2T8[:, :, t * P:(t + 1) * P],
                batch[:, 0:512].rearrange("p (c n) -> p c n", n=P))

        # ---- segment B: attention half 0, interleaved with Q half 1 -------
        # Q half-1 psums ride the score pool (psC stays small).
        qslices = [(wqr0, 0), (wqr0, 1), (wqr0, 2), (wqr0, 3),
                   (wqr1, 4), (wqr1, 5), (wqr1, 6), (wqr1, 7)]
        attn_h0 = ahp.tile([P, 4, D], BF16, tag="ah", name="attn_h0")

        for p in range(9):
            if p < 8:
                pts = ptsA if p % 2 == 0 else ptsB
                scores_exp(pts, p, 0)
            if p < 8:
                wqr, co = qslices[p]
                kq_co(wqr, co, 1, qT, zqT, bqs, ISD, pool=scorep, tag="s")
            if p > 0:
                pv = ptsA if (p - 1) % 2 == 0 else ptsB
                for h in (2 * (p - 1), 2 * (p - 1) + 1):
                    av_head(pv, h, 0)
                    trans_head(h, attn_h0)
        zTp.release()
        wst.release()
        xz2 = tc.alloc_tile_pool(name="xz2", bufs=2, side="left")

        # ---- segments C/D: MLP half0 || attention half1, then MLP half1 ---
        mlpp = tc.alloc_tile_pool(name="mlpp", bufs=2, space="PSUM")
        wf = tc.alloc_tile_pool(name="wf", bufs=2, side="right")
        h1p = tc.alloc_tile_pool(name="h1p", bufs=1, side="left")
        h1f8 = h1p.tile([P, 8, 512], F8)        # ff chunks 0-7 (block 0)
        h1f = h1p.tile([P, 24, 512], BF16)      # ff chunks 8-31
        xcp = tc.alloc_tile_pool(name="xcp", bufs=3, side="left")
        tailp = tc.alloc_tile_pool(name="tail", bufs=3, side="left")

        # prefetch the first MLP weight blocks under LN2/attention
        w1_tiles = {}
        w2_tiles = {}

        def w1_load(b):
            w18r = wf.tile([P, 4, 1024], F8, tag="w18r", name="w18r")
            nc.sync.dma_start(
                w18r[:], w18_d.rearrange("(c p) n -> p c n", p=P)[
                    :, :, b * 1024:(b + 1) * 1024])
            w1r = wf.tile([P, 4, 1024], BF16, tag="w1r", name="w1r")
            nc.sync.dma_start(
                w1r[:], w1_d.rearrange("(c p) n -> p c n", p=P)[
                    :, :, b * 1024:(b + 1) * 1024])
            return w18r, w1r

        def w2_load(co):
            w2co8 = wf.tile([P, 8, P], F8, tag="w2co8", name="w2co8")
            nc.sync.dma_start(
                w2co8[:], w28_d.rearrange("(f p) n -> p f n", p=P)[
                    :, :, co * P:(co + 1) * P])
            w2co = wf.tile([P, 24, P], BF16, tag="w2co", name="w2co")
            nc.sync.dma_start(
                w2co[:], w2_d.rearrange("(f p) n -> p f n", p=P)[
                    :, :, co * P:(co + 1) * P])
            return w2co8, w2co

        w1_tiles[0] = w1_load(0)
        w2_tiles[0] = w2_load(0)

        attn_h1 = ahp.tile([P, 4, D], BF16, tag="ah", name="attn_h1")

        # attention-half-1 work, sliced into steps the MLP emission pops.
        # Transposes lag their pair by one scores step (oa bufs=3 absorb
        # the lag) so the first steps don't touch attn_h1, which must wait
        # for the LN2-half0 reads of the shared attn buffer.
        attn1_steps = []
        trans_q = []
        for p in range(8):
            pts = ptsA if p % 2 == 0 else ptsB
            attn1_steps.append(
                lambda pts=pts, p=p: scores_exp(pts, p, 1))
            while len(trans_q) > 2:    # 2-pair lag: first 7 steps are
                attn1_steps.append(trans_q.pop(0))  # attn_h1-free
            for h in (2 * p, 2 * p + 1):
                dn = p == 7
                attn1_steps.append(
                    lambda pts=pts, h=h, dn=dn: av_head(pts, h, 1, dn))
                trans_q.append(
                    lambda h=h, dn=dn and h % 2 == 1:
                    trans_head(h, attn_h1, dn))
        attn1_steps.extend(trans_q)
        for t in range(4):
            attn1_steps.append(lambda t=t: ln2_tile(attn_h1, t, 1))

        a1 = iter(attn1_steps)

        def pop_attn(n):
            for _ in range(n):
                s = next(a1, None)
                if s is not None:
                    s()

        # LN2 half 0 with the first 7 attn-half-1 steps (scores p0..p2 and
        # their AVs — none touch attn_h1) keeping the PE fed
        pop_attn(1)
        for t in range(4):
            ln2_tile(attn_h0, t, 0)
            if t < 3:
                pop_attn(2)

        def h1_block(b, half, interleave=0):
            if b in w1_tiles:
                w18r, w1r = w1_tiles.pop(b)
            else:
                w18r, w1r = w1_load(b)
            for ft in range(8):
                hp = mlpp.tile([P, 512], F32, tag="m", name="hp")
                for j in range(2):    # dims 0:512 fp8, double-pumped
                    nc.tensor.matmul(
                        hp[:], w18r[:, 2 * j:2 * j + 2, ft * P:(ft + 1) * P],
                        z2T8[:, 2 * j:2 * j + 2, 0:512],
                        start=(j == 0), stop=False, perf_mode=DR)
                for kc in range(4):   # dims 512:1024 bf16
                    nc.tensor.matmul(
                        hp[:], w1r[:, kc, ft * P:(ft + 1) * P],
                        z2T[:, kc + 4, 0:512],
                        start=False, stop=(kc == 3))
                fc = b * 8 + ft
                h1dst = h1f8[:, ft, :] if b == 0 else h1f[:, fc - 8, :]
                nc.scalar.activation(
                    h1dst, hp[:], AF.Silu,
                    bias=b1s[:, fc:fc + 1], scale=1.0 / 32.0)
                if ft % 4 == 3:
                    pop_attn(interleave)
            return w18r, w1r

        def h2_block(co, half, interleave=0):
            if co in w2_tiles:
                w2co8, w2co = w2_tiles.pop(co)
            else:
                w2co8, w2co = w2_load(co)
            q0 = half * 512
            xc = xcp.tile([P, 512], F32, tag="xc", name="xc")
            nc.sync.dma_start(xc[:], xinT_d[co * P:(co + 1) * P, q0:q0 + 512])
            hp2 = mlpp.tile([P, 512], F32, tag="m", name="hp2")
            for j in range(4):        # ff chunks 0-7 fp8, double-pumped
                nc.tensor.matmul(
                    hp2[:], w2co8[:, 2 * j:2 * j + 2, :],
                    h1f8[:, 2 * j:2 * j + 2, :],
                    start=(j == 0), stop=False, perf_mode=DR)
            for fc in range(24):      # ff chunks 8-31 bf16
                nc.tensor.matmul(
                    hp2[:], w2co[:, fc, :], h1f[:, fc, :],
                    start=False, stop=(fc == 23))
                if fc % 12 == 11:
                    pop_attn(interleave)
            y = tailp.tile([P, 512], F32, tag="y", name="y")
            nc.vector.scalar_tensor_tensor(
                y[:], hp2[:], 1.0 / 64.0, xc[:], op0=ALU.mult, op1=ALU.add)
            nc.sync.dma_start(y_d[co * P:(co + 1) * P, q0:q0 + 512], y[:])
            return w2co8, w2co

        # half 0 MLP with attention half 1 interleaved (44 attn steps total)
        for b in range(4):
            w1r = h1_block(b, 0, interleave=2)
            if b == 3:
                w1_tiles[3] = w1r      # snake reuse for half 1
        for co in range(8):
            w2co = h2_block(co, 0, interleave=2)
            if co == 7:
                w2_tiles[7] = w2co
        pop_attn(99)  # drain any remaining attention-half-1 steps

        # half 1 MLP, snake order
        for b in (3, 2, 1, 0):
            h1_block(b, 1)
        for co in (7, 6, 5, 4, 3, 2, 1, 0):
            h2_block(co, 1)

        tailp.release()
        xcp.release()
        h1p.release()
        wf.release()
        mlpp.release()
        avtp.release()
        scorep.release()
        xz2.release()
        sbD.release()
        ahp.release()
        attnp.release()
        qkvp.release()
        cpool.release()

    nc.compile()
    return nc


def _prep_inputs(inputs):
    x = np.ascontiguousarray(np.asarray(inputs["x"], dtype=np.float32))
    kpm = np.asarray(inputs["key_pad_mask"]).astype(bool)
    wq = np.asarray(inputs["wq"], dtype=np.float32)
    wkv = np.asarray(inputs["wkv"], dtype=np.float32)
    w1 = np.asarray(inputs["w1"], dtype=np.float32)
    w2 = np.asarray(inputs["w2"], dtype=np.float32)
    bq = np.asarray(inputs["bq"], dtype=np.float32)
    bkv = np.asarray(inputs["bkv"], dtype=np.float32)
    b1 = np.asarray(inputs["b1"], dtype=np.float32)
    b2 = np.asarray(inputs["b2"], dtype=np.float32)
    ln1_g = np.asarray(inputs["ln1_g"], dtype=np.float32)
    ln1_b = np.asarray(inputs["ln1_b"], dtype=np.float32)
    ln2_g = np.asarray(inputs["ln2_g"], dtype=np.float32)
    ln2_b = np.asarray(inputs["ln2_b"], dtype=np.float32)

    # fold the LN affine transforms into the weights/biases (host-side):
    # (z*g + b) @ W + c == z @ (diag(g) W) + (b @ W + c)
    wq_f = ln1_g[:, None] * wq
    bq_f = ln1_b @ wq + bq
    wkv_f = ln1_g[:, None] * wkv
    bkv_f = ln1_b @ wkv + bkv
    w1_f = ln2_g[:, None] * w1
    b1_f = ln2_b @ w1 + b1

    def bf(v):
        return np.ascontiguousarray(v.astype(ml_dtypes.bfloat16))

    def dm(v):  # [D] -> [P, 8] dim-major chunk layout
        return np.ascontiguousarray(v.reshape(8, P).T)

    consts_base = np.concatenate([
        (bq_f * ISD).reshape(8, P).T,     # bqs
        dm(bkv_f[0:D]),                   # bkvk
        b1_f.reshape(32, P).T,            # b1s
        dm(b2),                           # b2s
    ], axis=1)                            # [P, 56]; keep appended per core

    # w1/w2 pre-scaled (32x / 64x) so their fp8-e4m3 rows sit in the
    # format's sweet spot (std ~1); silu / the output evac rescale back
    w1s = 32.0 * w1_f
    w2s = 64.0 * w2
    shared = {
        "wq": bf(wq_f),
        "wkv": bf(wkv_f),
        "w18": np.ascontiguousarray(
            w1s[0:D // 2].astype(ml_dtypes.float8_e4m3)),
        "w1": bf(w1s[D // 2:]),
        "w28": np.ascontiguousarray(
            w2s[0:D_FF // 4].astype(ml_dtypes.float8_e4m3)),
        "w2": bf(w2s[D_FF // 4:]),
        "bkvvb": np.ascontiguousarray(
            np.broadcast_to(bkv_f[D:2 * D], (P, D)).astype(np.float32)),
    }

    ki = np.arange(P)[:, None]   # key index within block (partition/row)
    qi = np.arange(P)[None, :]   # query index within block (free/col)
    tri = np.where(ki > qi, np.float32(EXPMASK), np.float32(KEEPVAL))
    keep = np.full((P, P), np.float32(KEEPVAL), dtype=np.float32)
    full = np.full((P, P), np.float32(EXPMASK), dtype=np.float32)

    in_maps = []
    for core in range(8):
        b, h = core // 2, core % 2
        perm = [2 * s + h for s in range(8)]
        xq = np.ascontiguousarray(
            x[b, 0:WINDOW * 2].reshape(16, P, D)[perm].reshape(WINDOW, D))
        xw = x[b, S - WINDOW:S]
        pad = kpm[b, S - WINDOW:S]
        m = dict(shared)
        m["xin"] = np.ascontiguousarray(
            np.concatenate([xq, xw], axis=0).astype(ml_dtypes.bfloat16))
        # residual carries the (folded) MLP output bias b2 per dim row
        m["xinT"] = np.ascontiguousarray(xq.T + b2[:, None])
        keepcol = (1.0 - pad.astype(np.float32)).reshape(8, P).T
        m["consts"] = np.ascontiguousarray(
            np.concatenate([consts_base, keepcol], axis=1))
        mE = tri if h == 0 else keep
        mO = full if h == 0 else tri
        m["masks"] = np.ascontiguousarray(
            np.concatenate([mE, mO], axis=1).astype(ml_dtypes.bfloat16))
        in_maps.append(m)
    return in_maps


def kernel(**inputs):
    from concourse.bass_utils import run_bass_kernel_spmd

    if "nc" not in _CACHE:
        _CACHE["nc"] = _build_program()
    nc = _CACHE["nc"]

    in_maps = _prep_inputs(inputs)
    trace = os.environ.get("KERNEL_TRACE", "0") == "1"
    res = run_bass_kernel_spmd(nc, in_maps, core_ids=list(range(8)),
                               trace=trace)
    if res.exec_time_ns is not None:
        print(f"HW exec time: {res.exec_time_ns} ns")
        _CACHE["exec_time_ns"] = res.exec_time_ns
    out = np.empty((B, S, D), dtype=np.float32)
    for core in range(8):
        b, h = core // 2, core % 2
        yT = res.results[core]["y"].T.reshape(8, P, D)
        dst = out[b, 0:WINDOW * 2].reshape(16, P, D)
        for s in range(8):
            dst[2 * s + h] = yT[s]
    return out
